# revision 1
# baseline (speedup 1.0000x reference)
"""Trainium2 Bass kernel for nn_Hard_Extract_Weight_Sum_Cluster.

Data-parallel over 8 cores: 4 examples per core (batch dim of x, 48 rows of
atten). Per example the kernel:
  1. Streams atten (12 heads x [512,512]) and computes exact column sums via a
     fixed-point split: coarse = round(a to 2^-11 grid) (fp16-exact), fine =
     a - coarse (|fine| <= 2^-12, fp16). Both are column-summed with fp16
     ones-matmuls into f32 PSUM; the coarse sum is bit-exact regardless of
     accumulation order, so the ranking below reproduces the f64-exact order.
  2. Extracts head diagonals with strided DMAs, sums over heads (PE).
  3. Ranks tokens with an exact two-float comparator:
     cnt_less[k] = #{j: (hi_j - hi_k) < (lo_k - lo_j)} via fused
     scalar_tensor_tensor with accum_out.
  4. Builds head/tail masks from cnt_less, computes ordinal positions with
     triangular-matrix matmuls, softmax weights for the tail, and:
     - gathers the 251 head rows of x with indirect DMA (offsets computed
       on-device by counting prefix ranks),
     - pools the 260 tail rows into 5 clusters with a weighted [5x512] matmul.
"""

import numpy as np

import concourse.bacc as bacc
import concourse.bass as bass
import concourse.mybir as mybir
from concourse.bass_utils import run_bass_kernel_spmd
from concourse.tile import TileContext

f32 = mybir.dt.float32
f16 = mybir.dt.float16
i32 = mybir.dt.int32
Alu = mybir.AluOpType
ActFn = mybir.ActivationFunctionType

B, S, D, H = 32, 512, 768, 12
N_CORES = 8
EX = B // N_CORES          # 4 examples per core
P = 128
NC_CHUNK = S // P          # 4 row-blocks per head matrix
WIDE = S * NC_CHUNK        # 2048: one head = [128, 2048]
N_HEAD_OUT = 251           # CLS + 250 extracted tokens
# cnt_less thresholds (count of strictly-smaller among all 512 slots, CLS = -4)
#   head:  cnt >= 262   dropped: 261   tail: 1..260   CLS: 0


def build_nc():
    nc = bacc.Bacc()
    x_in = nc.declare_dram_parameter("x", [EX * S, D], f32, isOutput=False)
    at_in = nc.declare_dram_parameter("atten", [EX * H, S, S], f32, isOutput=False)
    ones_p_f16 = nc.declare_dram_parameter("c_ones_p_f16", [P, 1], f16, isOutput=False)
    ones_p_f32 = nc.declare_dram_parameter("c_ones_p_f32", [P, 1], f32, isOutput=False)
    ones_h = nc.declare_dram_parameter("c_ones_h", [H, 1], f32, isOutput=False)
    ones_r_f32 = nc.declare_dram_parameter("c_ones_r_f32", [1, P], f32, isOutput=False)
    ones_r_f16 = nc.declare_dram_parameter("c_ones_r_f16", [1, P], f16, isOutput=False)
    id4 = nc.declare_dram_parameter("c_id4", [4, 4], f32, isOutput=False)
    triu_sq = nc.declare_dram_parameter("c_triu", [P, P], f16, isOutput=False)
    ones_sq = nc.declare_dram_parameter("c_ones_sq", [P, P], f16, isOutput=False)
    tri_inc = nc.declare_dram_parameter("c_tri_inc", [P, NC_CHUNK * S], f16, isOutput=False)
    iota2 = nc.declare_dram_parameter("c_iota2", [P, 2], f32, isOutput=False)
    ones_1 = nc.declare_dram_parameter("c_ones_1", [1, 1], f32, isOutput=False)
    lowb = nc.declare_dram_parameter("c_lowb", [P, 5], f32, isOutput=False)
    highb = nc.declare_dram_parameter("c_highb", [P, 5], f32, isOutput=False)
    out = nc.declare_dram_parameter("out", [EX, 256, D], f32, isOutput=True)

    at_flat = at_in[:].rearrange("a b c -> (a b c)")

    with TileContext(nc) as tc:
        with tc.tile_pool(name="cst", bufs=1) as cst, \
             tc.tile_pool(name="big", bufs=4) as big, \
             tc.tile_pool(name="med", bufs=3) as med, \
             tc.tile_pool(name="sm", bufs=2) as sm, \
             tc.tile_pool(name="ps_acc", bufs=1, space="PSUM") as ps_acc, \
             tc.tile_pool(name="ps_big", bufs=2, space="PSUM") as ps_big, \
             tc.tile_pool(name="ps_sm", bufs=2, space="PSUM") as ps_sm:

            # ---- constants ----
            c_ones_p16 = cst.tile([P, 1], f16)
            nc.sync.dma_start(out=c_ones_p16, in_=ones_p_f16[:])
            c_ones_p32 = cst.tile([P, 1], f32)
            nc.sync.dma_start(out=c_ones_p32, in_=ones_p_f32[:])
            c_ones_h = cst.tile([H, 1], f32)
            nc.sync.dma_start(out=c_ones_h, in_=ones_h[:])
            c_ones_r32 = cst.tile([1, P], f32)
            nc.sync.dma_start(out=c_ones_r32, in_=ones_r_f32[:])
            c_ones_r16 = cst.tile([1, P], f16)
            nc.sync.dma_start(out=c_ones_r16, in_=ones_r_f16[:])
            c_id4 = cst.tile([4, 4], f32)
            nc.sync.dma_start(out=c_id4, in_=id4[:])
            c_triu = cst.tile([P, P], f16)
            nc.sync.dma_start(out=c_triu, in_=triu_sq[:])
            c_ones_sq = cst.tile([P, P], f16)
            nc.sync.dma_start(out=c_ones_sq, in_=ones_sq[:])
            c_tri = cst.tile([P, NC_CHUNK * S], f16)
            nc.sync.dma_start(out=c_tri, in_=tri_inc[:])
            c_iota2 = cst.tile([P, 2], f32)
            nc.sync.dma_start(out=c_iota2, in_=iota2[:])
            c_ones_1 = cst.tile([1, 1], f32)
            nc.sync.dma_start(out=c_ones_1, in_=ones_1[:])
            c_lowb = cst.tile([P, 5], f32)
            nc.sync.dma_start(out=c_lowb, in_=lowb[:])
            c_highb = cst.tile([P, 5], f32)
            nc.sync.dma_start(out=c_highb, in_=highb[:])

            # per-example accumulators in free layout (partition 0)
            hi_sb = [cst.tile([1, S], f32, name=f"hi_sb{b}") for b in range(EX)]
            lo_sb = [cst.tile([1, S], f32, name=f"lo_sb{b}") for b in range(EX)]
            dg_sb = [cst.tile([1, S], f32, name=f"dg_sb{b}") for b in range(EX)]

            # ================= stage A: stream + reduce =================
            for b in range(EX):
                hi_ps = ps_acc.tile([1, S], f32, tag="hi")
                lo_ps = ps_acc.tile([1, S], f32, tag="lo")
                diag_t = sm.tile([H, S], f32, tag="diag")
                for h in range(H):
                    base = (b * H + h) * S * S
                    nc.sync.dma_start(
                        out=diag_t[h:h + 1, :],
                        in_=at_flat[base:base + (S - 1) * (S + 1) + 1:S + 1]
                            .rearrange("(a b) -> a b", a=1))
                    a_t = big.tile([P, WIDE], f32, tag="a")
                    nc.sync.dma_start(
                        out=a_t.rearrange("p (k j) -> p k j", k=NC_CHUNK),
                        in_=at_in[b * H + h].rearrange("(k p) j -> p k j", p=P))
                    c_t = med.tile([P, WIDE], f16, tag="c")
                    nc.gpsimd.tensor_scalar(c_t, a_t, 4096.0, 4096.0,
                                            op0=Alu.add, op1=Alu.subtract)
                    f_t = med.tile([P, WIDE], f16, tag="f")
                    nc.vector.tensor_tensor(out=f_t, in0=a_t, in1=c_t,
                                            op=Alu.subtract)
                    for k in range(NC_CHUNK):
                        first = (h == 0 and k == 0)
                        last = (h == H - 1 and k == NC_CHUNK - 1)
                        nc.tensor.matmul(hi_ps, lhsT=c_ones_p16,
                                         rhs=c_t[:, k * S:(k + 1) * S],
                                         start=first, stop=last,
                                         skip_group_check=True)
                        nc.tensor.matmul(lo_ps, lhsT=c_ones_p16,
                                         rhs=f_t[:, k * S:(k + 1) * S],
                                         start=first, stop=last,
                                         skip_group_check=True)
                nc.scalar.copy(hi_sb[b], hi_ps)
                nc.scalar.copy(lo_sb[b], lo_ps)
                dg_ps = ps_sm.tile([1, S], f32, tag="scr")
                nc.tensor.matmul(dg_ps, lhsT=c_ones_h, rhs=diag_t,
                                 start=True, stop=True)
                nc.scalar.copy(dg_sb[b], dg_ps)

            # ================= finalize hi/lo =================
            lo1_sb = [cst.tile([1, S], f32, name=f"lo1_sb{b}") for b in range(EX)]
            for b in range(EX):
                nc.vector.tensor_tensor(out=lo1_sb[b], in0=lo_sb[b],
                                        in1=dg_sb[b], op=Alu.subtract)
                nc.vector.memset(hi_sb[b][:, 0:1], -4.0)
                nc.vector.memset(lo1_sb[b][:, 0:1], 0.0)

            # transposes via ones[1,1] matmul: [1,128] slice -> [128,1] col 4c+b
            hiT_ps = ps_sm.tile([P, 4 * EX], f32, tag="scr")
            loT_ps = ps_sm.tile([P, 4 * EX], f32, tag="scr2")
            for c in range(NC_CHUNK):
                for b in range(EX):
                    col = 4 * c + b
                    nc.tensor.matmul(hiT_ps[:, col:col + 1],
                                     lhsT=hi_sb[b][0:1, c * P:(c + 1) * P],
                                     rhs=c_ones_1, start=True, stop=True)
                    nc.tensor.matmul(loT_ps[:, col:col + 1],
                                     lhsT=lo1_sb[b][0:1, c * P:(c + 1) * P],
                                     rhs=c_ones_1, start=True, stop=True)
            hiT = cst.tile([P, 4 * EX], f32)
            nc.scalar.copy(hiT, hiT_ps)
            loT = cst.tile([P, 4 * EX], f32)
            nc.scalar.copy(loT, loT_ps)

            # softmax numerators (tail weights), all examples at once
            s_t = cst.tile([P, 4 * EX], f32)
            nc.vector.tensor_tensor(out=s_t, in0=hiT, in1=loT, op=Alu.add)
            e_t = cst.tile([P, 4 * EX], f32)
            bias_t = cst.tile([P, 1], f32)
            nc.vector.memset(bias_t, -256.0)
            nc.scalar.activation(e_t, s_t, ActFn.Exp, bias=bias_t[:, 0:1],
                                 scale=1.0 / 12.0)

            cnt = cst.tile([P, 4 * EX], f32)
            m_ext = cst.tile([P, 4 * EX], f16)
            m_tail = cst.tile([P, 4 * EX], f16)
            e_m = cst.tile([P, 4 * EX], f32)

            for b in range(EX):
                # broadcast hi/lo rows across partitions
                bch_ps = ps_big.tile([P, S], f32, tag="bc")
                nc.tensor.matmul(bch_ps, lhsT=c_ones_r32,
                                 rhs=hi_sb[b], start=True, stop=True)
                bch = med.tile([P, S], f32, tag="bch")
                nc.scalar.copy(bch, bch_ps)
                bcl_ps = ps_big.tile([P, S], f32, tag="bc")
                nc.tensor.matmul(bcl_ps, lhsT=c_ones_r32,
                                 rhs=lo1_sb[b], start=True, stop=True)
                bcl = med.tile([P, S], f32, tag="bcl")
                nc.scalar.copy(bcl, bcl_ps)

                # exact two-float rank: cnt_less[k] = sum_j (v_j < v_k)
                for c in range(NC_CHUNK):
                    col = 4 * c + b
                    F_t = sm.tile([P, S], f32, tag="F")
                    nc.gpsimd.tensor_scalar(F_t, bcl, -1.0, loT[:, col:col + 1],
                                            op0=Alu.mult, op1=Alu.add)
                    scr_t = sm.tile([P, S], f16, tag="scr")
                    nc.vector.scalar_tensor_tensor(
                        out=scr_t, in0=bch, scalar=hiT[:, col:col + 1], in1=F_t,
                        op0=Alu.subtract, op1=Alu.is_lt,
                        accum_out=cnt[:, col:col + 1])

            # masks from cnt_less
            nc.vector.tensor_scalar(m_ext, cnt, 261.5, None, op0=Alu.is_ge)
            mta = sm.tile([P, 4 * EX], f16, tag="mta")
            nc.vector.tensor_scalar(mta, cnt, 0.5, None, op0=Alu.is_gt)
            mtb = sm.tile([P, 4 * EX], f16, tag="mtb")
            nc.vector.tensor_scalar(mtb, cnt, 260.5, None, op0=Alu.is_lt)
            nc.vector.tensor_tensor(out=m_tail, in0=mta, in1=mtb, op=Alu.mult)
            # CLS (k=0, chunk 0, partition 0) joins the extract set
            nc.vector.memset(m_ext[0:1, 0:EX], 1.0)
            nc.vector.tensor_tensor(out=e_m, in0=e_t, in1=m_tail, op=Alu.mult)

            # tail normalization: Z per example, then 1/(53 Z) per partition
            z_ps = ps_sm.tile([1, 4 * EX], f32, tag="scr")
            nc.tensor.matmul(z_ps, lhsT=c_ones_p32, rhs=e_m, start=True, stop=True)
            z_sb = sm.tile([1, 4 * EX], f32, tag="zsb")
            nc.scalar.copy(z_sb, z_ps)
            z4 = sm.tile([1, EX], f32, tag="z4")
            nc.vector.tensor_reduce(
                z4, z_sb.rearrange("a (c b) -> a b c", b=EX),
                axis=mybir.AxisListType.X, op=Alu.add)
            rz4 = sm.tile([1, EX], f32, tag="rz4")
            nc.vector.reciprocal(rz4, z4)

            for b in range(EX):
                # P_ext inclusive prefix (free layout) -> src offsets
                pe_ps = ps_sm.tile([1, S], f32, tag="scr")
                for c in range(NC_CHUNK):
                    nc.tensor.matmul(pe_ps, lhsT=m_ext[:, 4 * c + b:4 * c + b + 1],
                                                     rhs=c_tri[:, c * S:(c + 1) * S],
                                     start=(c == 0),
                                     stop=(c == NC_CHUNK - 1),
                                     skip_group_check=True)
                pe_sb = sm.tile([1, S], f16, tag="pesb")
                nc.scalar.copy(pe_sb, pe_ps)
                bcp_ps = ps_big.tile([P, S], f32, tag="bc")
                nc.tensor.matmul(bcp_ps, lhsT=c_ones_r16, rhs=pe_sb,
                                 start=True, stop=True)
                bcp_sb = med.tile([P, S], f32, tag="bcpsb")
                nc.scalar.copy(bcp_sb, bcp_ps)
                src_f = sm.tile([P, 2], f32, tag="srcf")
                for rc in range(2):
                    scr2 = sm.tile([P, S], f16, tag="scr2")
                    nc.vector.scalar_tensor_tensor(
                        out=scr2, in0=bcp_sb, scalar=c_iota2[:, rc:rc + 1],
                        in1=bcp_sb, op0=Alu.is_le, op1=Alu.bypass,
                        accum_out=src_f[:, rc:rc + 1])
                src_i = sm.tile([P, 2], i32, tag="srci")
                nc.vector.tensor_scalar(src_i, src_f, float(b * S), None,
                                        op0=Alu.add)
                # gather head rows of x -> out rows 0..250
                g0 = med.tile([P, D], f32, tag="g0")
                nc.gpsimd.indirect_dma_start(
                    out=g0, out_offset=None, in_=x_in[:],
                    in_offset=bass.IndirectOffsetOnAxis(ap=src_i[:, 0:1], axis=0))
                nc.sync.dma_start(out=out[b, 0:P, :], in_=g0)
                g1 = med.tile([P, D], f32, tag="g1")
                nc.gpsimd.indirect_dma_start(
                    out=g1[0:N_HEAD_OUT - P, :], out_offset=None, in_=x_in[:],
                    in_offset=bass.IndirectOffsetOnAxis(ap=src_i[0:N_HEAD_OUT - P, 1:2],
                                                        axis=0))
                nc.sync.dma_start(out=out[b, P:N_HEAD_OUT, :],
                                  in_=g1[0:N_HEAD_OUT - P, :])

                # tail positions in partition layout (inclusive prefix)
                tp_ps = ps_sm.tile([P, NC_CHUNK], f32, tag="scr")
                for c in range(NC_CHUNK):
                    for cc in range(c + 1):
                        nc.tensor.matmul(
                            tp_ps[:, c:c + 1],
                            lhsT=(c_triu if cc == c else c_ones_sq),
                            rhs=m_tail[:, 4 * cc + b:4 * cc + b + 1],
                            start=(cc == 0), stop=(cc == c),
                            skip_group_check=True)
                tp_sb = sm.tile([P, NC_CHUNK], f32, tag="tpsb")
                nc.scalar.copy(tp_sb, tp_ps)

                # 1/(53 Z_b) broadcast across partitions
                rzb_ps = ps_sm.tile([P, 1], f32, tag="scr2")
                nc.tensor.matmul(rzb_ps, lhsT=c_ones_r32, rhs=rz4[0:1, b:b + 1],
                                 start=True, stop=True)
                rz53 = sm.tile([P, 1], f32, tag="rz53")
                nc.vector.tensor_scalar(rz53, rzb_ps, 1.0 / 53.0, None,
                                        op0=Alu.mult)

                # weighted cluster matmul
                x_t = big.tile([P, NC_CHUNK * D], f32, tag="x")
                nc.sync.dma_start(
                    out=x_t.rearrange("p (k d) -> p k d", k=NC_CHUNK),
                    in_=x_in[b * S:(b + 1) * S, :].rearrange("(k p) d -> p k d", p=P))
                cl_a = ps_sm.tile([5, S], f32, tag="scr")
                cl_b = ps_sm.tile([5, D - S], f32, tag="scr2")
                for c in range(NC_CHUNK):
                    o2 = sm.tile([P, 5], f32, tag="o2")
                    nc.vector.tensor_scalar(o2, c_highb, tp_sb[:, c:c + 1], None,
                                            op0=Alu.is_gt)
                    oh = sm.tile([P, 5], f32, tag="oh")
                    nc.vector.scalar_tensor_tensor(
                        out=oh, in0=c_lowb, scalar=tp_sb[:, c:c + 1], in1=o2,
                        op0=Alu.is_lt, op1=Alu.mult)
                    wq = sm.tile([P, 5], f32, tag="wq")
                    nc.vector.tensor_scalar(
                        wq, oh, e_m[:, 4 * c + b:4 * c + b + 1], rz53[:, 0:1],
                        op0=Alu.mult, op1=Alu.mult)
                    nc.tensor.matmul(cl_a, lhsT=wq, rhs=x_t[:, c * D:c * D + S],
                                     start=(c == 0), stop=(c == NC_CHUNK - 1),
                                     skip_group_check=True)
                    nc.tensor.matmul(cl_b, lhsT=wq, rhs=x_t[:, c * D + S:(c + 1) * D],
                                     start=(c == 0), stop=(c == NC_CHUNK - 1),
                                     skip_group_check=True)
                cl_sb = sm.tile([5, D], f32, tag="clsb")
                nc.scalar.copy(cl_sb[:, 0:S], cl_a)
                nc.scalar.copy(cl_sb[:, S:D], cl_b)
                nc.sync.dma_start(out=out[b, N_HEAD_OUT:256, :], in_=cl_sb)

    nc.compile()
    return nc


_NC_CACHE = {}


def _consts():
    tri = np.zeros((P, NC_CHUNK * S), np.float16)
    for c in range(NC_CHUNK):
        for p in range(P):
            tri[p, c * S + c * P + p:(c + 1) * S] = 1.0
    iota2 = (np.arange(P, dtype=np.float32)[:, None]
             + np.array([0.0, 128.0], np.float32)[None, :])
    lowb = np.tile((53.0 * np.arange(5, dtype=np.float32) + 0.5)[None, :], (P, 1))
    highb = np.tile((53.0 * np.arange(5, dtype=np.float32) + 53.5)[None, :], (P, 1))
    return {
        "c_ones_p_f16": np.ones((P, 1), np.float16),
        "c_ones_p_f32": np.ones((P, 1), np.float32),
        "c_ones_h": np.ones((H, 1), np.float32),
        "c_ones_r_f32": np.ones((1, P), np.float32),
        "c_ones_r_f16": np.ones((1, P), np.float16),
        "c_id4": np.eye(4, dtype=np.float32),
        "c_triu": np.triu(np.ones((P, P))).astype(np.float16),
        "c_ones_sq": np.ones((P, P), np.float16),
        "c_tri_inc": tri,
        "c_iota2": iota2,
        "c_ones_1": np.ones((1, 1), np.float32),
        "c_lowb": lowb,
        "c_highb": highb,
    }


def kernel(x: np.ndarray, atten: np.ndarray, trace: bool = False):
    if "nc" not in _NC_CACHE:
        _NC_CACHE["nc"] = build_nc()
    nc = _NC_CACHE["nc"]
    x = np.ascontiguousarray(np.asarray(x, np.float32))
    atten = np.ascontiguousarray(np.asarray(atten, np.float32))
    consts = _consts()
    in_maps = []
    for ci in range(N_CORES):
        in_maps.append({
            "x": x[ci * EX:(ci + 1) * EX].reshape(EX * S, D),
            "atten": atten[ci * EX * H:(ci + 1) * EX * H],
            **consts,
        })
    res = run_bass_kernel_spmd(nc, in_maps, list(range(N_CORES)), trace=trace)
    _NC_CACHE["last_res"] = res
    out = np.concatenate([res.results[ci]["out"] for ci in range(N_CORES)], axis=0)
    if trace:
        return out, res
    return out

